# revision 17
# baseline (speedup 1.0000x reference)
"""Trainium2 Bass kernel for an edge-MLP GNN block.

  v0 = x_node[edge_index[:, 0]]          # [E, D] gather
  v1 = x_node[edge_index[:, 1]]          # [E, D] gather
  h  = relu(concat([v0, v1, x_edge]) @ W1 + b1)
  out = h @ W2 + b2                      # [E, D]

Sharding: edges are split evenly across 8 NeuronCores; x_node and the MLP
weights are replicated.  No cross-core communication.

Gather: the SWDGE `dma_gather` extended instruction fetches one 512B node row
per index, thousands of rows per instruction (descriptor-generation is the
per-instruction cost, ~1us + 0.34ns/row).  Its indices are int16, so the host
partitions each core's edges into 4 classes by (src >= 32768, dst >= 32768)
and each block gathers from a base-offset view of x_node; the host permutes
edges into class-contiguous order and inverse-permutes the output.

Per-core layout: edges are processed in blocks of 128*k edges.  dma_gather
writes row i of a block to partition i%128, column-block i//128.  The host
pre-permutes the x_edge shard (and inverse-permutes the output) so the device
x_edge load and output store stay fully contiguous per partition.

On-chip per 512-edge group: PE transposes the three [128e x 128f] operand
tiles into feature-major form, mm1 accumulates the three K-chunks of W1 into
PSUM ([d1 x 512e]), ACT applies bias+ReLU, mm2 multiplies each 128-edge chunk
by W2 (output lands edge-major, no output transpose needed), and DVE adds b2
while copying PSUM->SBUF.
"""

import os
import sys

sys.path.insert(0, "/opt/trn_rl_repo")

import numpy as np

import concourse.bacc as bacc
import concourse.bass as bass
import concourse.mybir as mybir
import concourse.tile as tile
from concourse.bass_utils import run_bass_kernel_spmd

P = 128
D = 128
N_NODES = 50000
E_TOTAL = 500000
N_CORES = 8
EC = E_TOTAL // N_CORES  # 62500 edges per core
TH = 32768               # int16 index ceiling; class split threshold

K_FULL = 16              # column-blocks (edges per partition) per full block
GROUP = 4                # column-blocks per matmul group (N = 512)

f32 = mybir.dt.float32
f32r = mybir.dt.float32r
bf16 = mybir.dt.bfloat16
i16 = mybir.dt.int16

# Compute dtype for the matmuls: "f32" (exact, 4 cyc/row), "f32r" (single-pass
# fp32, 1 cyc/row at N>=256), "bf16" (1 cyc/row; activations/weights rounded).
MM_MODE = os.environ.get("KMM_DTYPE", "f32r")
# Gather node rows in bf16 (host-cast x_node): halves gather DMA bytes and
# transpose PE cycles at ~2e-3 output error (vs ~2e-4).
GATHER_BF16 = os.environ.get("KGATHER_BF16", "0") == "1"
g_dt = bf16 if GATHER_BF16 else f32
G_ELEM = 64 if GATHER_BF16 else 128  # dma_gather elem_size: bf16 rows read as 128 x 2B


def _class_blocks(cap):
    """Block sizes (k = column-blocks) for one class capacity (multiple of 128)."""
    n = cap // P
    out = [K_FULL] * (n // K_FULL)
    if n % K_FULL:
        out.append(n % K_FULL)
    return out


def _build_module(blocks):
    """blocks: list of (k, hi0, hi1) — identical on every core."""
    sb_dt = {"bf16": bf16, "f32r": f32r, "f32": f32}[MM_MODE]
    ecp = sum(k for k, _, _ in blocks) * P
    idx_tot = sum(P * 8 * k for k, _, _ in blocks)  # per-block [P, 8k] int16

    nc = bacc.Bacc("TRN2", num_swdge_queues=4)
    xn = nc.dram_tensor("x_node", [N_NODES, D], g_dt, kind="ExternalInput")
    xet_d = nc.dram_tensor("x_edgeT", [ecp * D], f32r, kind="ExternalInput")
    i0 = nc.dram_tensor("idx0", [idx_tot], i16, kind="ExternalInput")
    i1 = nc.dram_tensor("idx1", [idx_tot], i16, kind="ExternalInput")
    w1 = nc.dram_tensor("W1", [3 * D, D], f32, kind="ExternalInput")
    b1 = nc.dram_tensor("b1", [D], f32, kind="ExternalInput")
    w2 = nc.dram_tensor("W2", [D, D], f32, kind="ExternalInput")
    b2r = nc.dram_tensor("b2r", [P, D], f32, kind="ExternalInput")
    out = nc.dram_tensor("out", [ecp, D], f32, kind="ExternalOutput")

    xn_hi = xn[TH:, :]

    from concourse.masks import make_identity

    with (
        tile.TileContext(nc) as tc,
        tc.tile_pool(name="const", bufs=1) as cpool,
        tc.tile_pool(name="big", bufs=3) as big,
        tc.tile_pool(name="tsp", bufs=2) as tsp,
        tc.tile_pool(name="psT", bufs=1, space="PSUM") as psT,
        tc.tile_pool(name="psH", bufs=2, space="PSUM") as psH,
        tc.tile_pool(name="psO", bufs=2, space="PSUM") as psO,
    ):
        ident = cpool.tile([P, P], g_dt)
        make_identity(nc, ident[:])
        mm01_dt = bf16 if GATHER_BF16 else sb_dt

        # W1 as 3 K-chunks: chunk c = W1[128c:128c+128, :] -> w1_sb[:, c, :]
        # Staged through fp32 then copied so the compute-dtype rounding is
        # done by an engine write (required for FP32R).
        w1_sb = cpool.tile([P, 3, D], sb_dt)
        w2_sb = cpool.tile([P, D], sb_dt)
        if MM_MODE == "f32":
            nc.sync.dma_start(out=w1_sb[:], in_=w1[:].rearrange("(c p) d -> p c d", p=P))
            nc.sync.dma_start(out=w2_sb[:], in_=w2[:])
        else:
            w1_st = cpool.tile([P, 3, D], f32)
            w2_st = cpool.tile([P, D], f32)
            nc.sync.dma_start(out=w1_st[:], in_=w1[:].rearrange("(c p) d -> p c d", p=P))
            nc.sync.dma_start(out=w2_st[:], in_=w2[:])
            nc.any.tensor_copy(w1_sb[:], w1_st[:])
            nc.any.tensor_copy(w2_sb[:], w2_st[:])
        w1_01 = cpool.tile([P, 2, D], mm01_dt)
        if MM_MODE == "f32":
            w1_f32src = w1_sb
        else:
            w1_f32src = w1_st
        nc.any.tensor_copy(w1_01[:], w1_f32src[:, 0:2, :])
        b1_sb = cpool.tile([P, 1], f32)
        nc.sync.dma_start(out=b1_sb[:], in_=b1[:].unsqueeze(1))
        # b2 replicated across partitions, prepared host-side.
        b2bc = cpool.tile([P, D], f32)
        nc.sync.dma_start(out=b2bc[:], in_=b2r[:])

        eoff = 0
        ioff = 0
        goff = 0
        gq = 0
        for kb, hi0, hi1 in blocks:
            BL = P * kb
            S = 8 * kb  # int16 index free dim: ceil(BL/16)
            v0 = big.tile([P, kb * D], g_dt, tag="v0")
            v1 = big.tile([P, kb * D], g_dt, tag="v1")
            xbt = big.tile([P, kb * D], sb_dt, tag="xbt")
            ot = big.tile([P, kb * D], f32, tag="ot")
            ix0 = big.tile([P, S], i16, tag="ix0")
            ix1 = big.tile([P, S], i16, tag="ix1")

            nc.sync.dma_start(out=ix0[:], in_=i0[ioff : ioff + P * S].rearrange("(p s) -> p s", p=P))
            nc.sync.dma_start(out=ix1[:], in_=i1[ioff : ioff + P * S].rearrange("(p s) -> p s", p=P))
            nc.sync.dma_start(
                out=xbt[:],
                in_=xet_d[goff : goff + BL * D].rearrange("(p n) -> p n", p=P),
            )
            # dma_gather crashes the device above ~1024 indices per
            # instruction (descriptor-ring capacity); chunk by 8 col-blocks.
            for vt, ixt, hi in ((v0, ix0, hi0), (v1, ix1, hi1)):
                for k0 in range(0, kb, 8):
                    kw = min(8, kb - k0)
                    nc.gpsimd.dma_gather(
                        vt[:, k0 * D : (k0 + kw) * D].rearrange("p (k d) -> p k d", d=D),
                        xn_hi if hi else xn[:, :],
                        ixt[:, 8 * k0 : 8 * (k0 + kw)],
                        P * kw,
                        P * kw,
                        D,
                        queue_num=gq % 4,
                    )
                    gq += 1

            for g0 in range(0, kb, GROUP):
                wid = min(GROUP, kb - g0)
                N = wid * P

                ph = psH.tile([P, GROUP * P], f32, tag="ph")
                for kind, src in enumerate((v0, v1)):
                    pv = psT.tile([P, GROUP * P], g_dt, tag=f"pv{kind}", name=f"pv{kind}")
                    tv = tsp.tile([P, GROUP * P], mm01_dt, tag=f"tv{kind}", name=f"tv{kind}")
                    for c in range(wid):
                        j = g0 + c
                        nc.tensor.transpose(
                            out=pv[:, c * P : (c + 1) * P],
                            in_=src[:, j * D : (j + 1) * D],
                            identity=ident[:],
                        )
                    # Fixed engine per kind: v0 copy on ACT, v1 on DVE.
                    if kind == 0:
                        nc.scalar.activation(
                            out=tv[:, :N], in_=pv[:, :N],
                            func=mybir.ActivationFunctionType.Copy,
                        )
                    else:
                        nc.vector.tensor_copy(tv[:, :N], pv[:, :N])
                    nc.tensor.matmul(
                        ph[:, :N],
                        lhsT=w1_01[:, kind, :],
                        rhs=tv[:, :N],
                        start=(kind == 0),
                        stop=False,
                    )
                nc.tensor.matmul(
                    ph[:, :N],
                    lhsT=w1_sb[:, 2, :],
                    rhs=xbt[:, g0 * P : g0 * P + N],
                    start=False,
                    stop=True,
                )

                h = tsp.tile([P, GROUP * P], sb_dt, tag="h")
                nc.scalar.activation(
                    out=h[:, :N],
                    in_=ph[:, :N],
                    func=mybir.ActivationFunctionType.Relu,
                    bias=b1_sb[:, 0:1],
                )

                po = psO.tile([P, GROUP * P], f32, tag="po")
                for c in range(wid):
                    nc.tensor.matmul(
                        po[:, c * P : (c + 1) * P],
                        lhsT=h[:, c * P : (c + 1) * P],
                        rhs=w2_sb[:],
                        start=True,
                        stop=True,
                    )

                # out = po + b2 (broadcast along edges), PSUM -> SBUF
                nc.vector.tensor_tensor(
                    out=ot[:, g0 * D : g0 * D + N].rearrange("p (c d) -> p c d", d=P),
                    in0=po[:, :N].rearrange("p (c d) -> p c d", d=P),
                    in1=b2bc[:].unsqueeze(1).to_broadcast([P, wid, P]),
                    op=mybir.AluOpType.add,
                )

            nc.scalar.dma_start(
                out=out[eoff : eoff + BL, :].rearrange("(p k) d -> p (k d)", p=P),
                in_=ot[:],
            )
            eoff += BL
            ioff += P * S
            goff += BL * D

    nc.compile()
    return nc


def _pack_idx(vals):
    """[BL] int16 values -> flat [P*8k] device layout: idx i at
    (partition i%16 replicated 8x, free slot i//16), partition-major."""
    BL = vals.shape[0]
    S = BL // 16
    t16 = vals.reshape(S, 16).T  # [16, S]
    return np.tile(t16, (8, 1)).ravel()  # [128, S] -> flat p-major


def _plan_and_pack(x_edge, ei):
    """Host-side: class-partition, permute, build per-core device arrays.

    Returns (blocks, per-core input dicts, per-core (perm, valid), ecp)."""
    v0 = ei[:, 0].astype(np.int64)
    v1 = ei[:, 1].astype(np.int64)
    cls = (v0 >= TH) * 2 + (v1 >= TH)

    per_core = []
    for c in range(N_CORES):
        sl = slice(c * EC, (c + 1) * EC)
        per_core.append((v0[sl], v1[sl], cls[sl]))

    caps = []
    for cl in range(4):
        cnt = max(int((pc[2] == cl).sum()) for pc in per_core)
        caps.append(-(-max(cnt, 1) // P) * P)

    blocks = []
    for cl in range(4):
        hi0, hi1 = bool(cl & 2), bool(cl & 1)
        blocks.extend((k, hi0, hi1) for k in _class_blocks(caps[cl]))
    ecp = sum(k for k, _, _ in blocks) * P

    core_data = []
    core_asm = []
    for c in range(N_CORES):
        cv0, cv1, ccls = per_core[c]
        xe_core = x_edge[c * EC : (c + 1) * EC]

        # permuted order: class-grouped, padded per class
        perm = np.full(ecp, -1, dtype=np.int64)  # padded-perm pos -> core-local edge
        off = 0
        for cl in range(4):
            ids = np.nonzero(ccls == cl)[0]
            # ascending v0 within the class: gather addresses mostly
            # monotonic -> better HBM locality for the v0 gather
            ids = ids[np.argsort(cv0[ids], kind="stable")]
            perm[off : off + len(ids)] = ids
            off += caps[cl]
        valid = perm >= 0
        pidx = np.where(valid, perm, 0)

        pos = np.arange(ecp)
        hi0_mask = pos >= caps[0] + caps[1]
        hi1_mask = ((pos >= caps[0]) & (pos < caps[0] + caps[1])) | (
            pos >= caps[0] + caps[1] + caps[2]
        )
        pv0 = np.where(valid, cv0[pidx] - np.where(hi0_mask, TH, 0), 0).astype(np.int16)
        pv1 = np.where(valid, cv1[pidx] - np.where(hi1_mask, TH, 0), 0).astype(np.int16)

        xe_perm = np.zeros((ecp, D), np.float32)
        xe_perm[valid] = xe_core[perm[valid]]

        # xeT: per matmul group, the [N, D] edge slab transposed to [D, N]
        # (feature-major) so the device loads mm1's rhs directly.
        xeT_parts = []
        i0_parts = []
        i1_parts = []
        eoff = 0
        for kb, _, _ in blocks:
            BL = P * kb
            slab = xe_perm[eoff : eoff + BL]  # [BL, D], edge i = c*128+p
            xeT_parts.append(np.ascontiguousarray(slab.T).ravel())
            i0_parts.append(_pack_idx(pv0[eoff : eoff + BL]))
            i1_parts.append(_pack_idx(pv1[eoff : eoff + BL]))
            eoff += BL

        core_data.append(
            {
                "x_edgeT": np.concatenate(xeT_parts),
                "idx0": np.concatenate(i0_parts),
                "idx1": np.concatenate(i1_parts),
            }
        )
        core_asm.append((perm, valid))

    return blocks, core_data, core_asm, ecp


_module_cache = {}


def _get_module(blocks_key):
    if blocks_key not in _module_cache:
        _module_cache[blocks_key] = _build_module(list(blocks_key))
    return _module_cache[blocks_key]


def run(inputs, trace=False):
    """Run on 8 cores. Returns (full_output [E, D] fp32, BassKernelResults)."""
    x_node = np.ascontiguousarray(np.asarray(inputs["x_node"], dtype=np.float32))
    x_edge = np.ascontiguousarray(np.asarray(inputs["x_edge"], dtype=np.float32))
    ei = np.asarray(inputs["edge_index"])
    W1 = np.ascontiguousarray(np.asarray(inputs["W1"], dtype=np.float32))
    b1 = np.ascontiguousarray(np.asarray(inputs["b1"], dtype=np.float32))
    W2 = np.ascontiguousarray(np.asarray(inputs["W2"], dtype=np.float32))
    b2 = np.ascontiguousarray(np.asarray(inputs["b2"], dtype=np.float32))
    b2r_host = np.ascontiguousarray(np.broadcast_to(b2, (P, D)).copy())

    blocks, core_data, core_asm, ecp = _plan_and_pack(x_edge, ei)
    nc = _get_module(tuple(blocks))

    if GATHER_BF16:
        import ml_dtypes

        x_node = x_node.astype(ml_dtypes.bfloat16)

    in_maps = []
    for c in range(N_CORES):
        in_maps.append(
            {
                "x_node": x_node,
                "x_edgeT": core_data[c]["x_edgeT"],
                "idx0": core_data[c]["idx0"],
                "idx1": core_data[c]["idx1"],
                "W1": W1,
                "b1": b1,
                "W2": W2,
                "b2r": b2r_host,
            }
        )

    res = run_bass_kernel_spmd(nc, in_maps, core_ids=list(range(N_CORES)), trace=trace)

    full = np.empty((E_TOTAL, D), np.float32)
    eoffs = np.cumsum([0] + [P * k for k, _, _ in blocks])
    for c in range(N_CORES):
        dev_out = res.results[c]["out"]
        perm, valid = core_asm[c]
        res_perm = np.empty((ecp, D), np.float32)
        for bi, (kb, _, _) in enumerate(blocks):
            a, b = int(eoffs[bi]), int(eoffs[bi + 1])
            res_perm[a:b] = (
                dev_out[a:b].reshape(P, kb, D).transpose(1, 0, 2).reshape(b - a, D)
            )
        out_core = full[c * EC : (c + 1) * EC]
        out_core[perm[valid]] = res_perm[valid]
    return full, res


def kernel(**inputs):
    out, _ = run(inputs, trace=False)
    return out


# revision 19
# speedup vs baseline: 1.4259x; 1.4259x over previous
"""Trainium2 Bass kernel for an edge-MLP GNN block.

  v0 = x_node[edge_index[:, 0]]          # [E, D] gather
  v1 = x_node[edge_index[:, 1]]          # [E, D] gather
  h  = relu(concat([v0, v1, x_edge]) @ W1 + b1)
  out = h @ W2 + b2                      # [E, D]

Sharding: edges are split evenly across 8 NeuronCores; x_node and the MLP
weights are replicated.  No cross-core communication.

Gather: the SWDGE `dma_gather` extended instruction fetches one 512B node row
per index, thousands of rows per instruction (descriptor-generation is the
per-instruction cost, ~1us + 0.34ns/row).  Its indices are int16, so the host
partitions each core's edges into 4 classes by (src >= 32768, dst >= 32768)
and each block gathers from a base-offset view of x_node; the host permutes
edges into class-contiguous order and inverse-permutes the output.

Per-core layout: edges are processed in blocks of 128*k edges.  dma_gather
writes row i of a block to partition i%128, column-block i//128.  The host
pre-permutes the x_edge shard (and inverse-permutes the output) so the device
x_edge load and output store stay fully contiguous per partition.

On-chip per 512-edge group: PE transposes the three [128e x 128f] operand
tiles into feature-major form, mm1 accumulates the three K-chunks of W1 into
PSUM ([d1 x 512e]), ACT applies bias+ReLU, mm2 multiplies each 128-edge chunk
by W2 (output lands edge-major, no output transpose needed), and DVE adds b2
while copying PSUM->SBUF.
"""

import os
import sys

sys.path.insert(0, "/opt/trn_rl_repo")

import numpy as np

import concourse.bacc as bacc
import concourse.bass as bass
import concourse.mybir as mybir
import concourse.tile as tile
from concourse.bass_utils import run_bass_kernel_spmd

P = 128
D = 128
N_NODES = 50000
E_TOTAL = 500000
N_CORES = 8
EC = E_TOTAL // N_CORES  # 62500 edges per core
TH = 32768               # int16 index ceiling; class split threshold

K_FULL = 16              # column-blocks (edges per partition) per full block
GROUP = 4                # column-blocks per matmul group (N = 512)

f32 = mybir.dt.float32
f32r = mybir.dt.float32r
bf16 = mybir.dt.bfloat16
i16 = mybir.dt.int16

# Compute dtype for the matmuls: "f32" (exact, 4 cyc/row), "f32r" (single-pass
# fp32, 1 cyc/row at N>=256), "bf16" (1 cyc/row; activations/weights rounded).
MM_MODE = os.environ.get("KMM_DTYPE", "f32r")
# Gather node rows in bf16 (host-cast x_node): halves gather DMA bytes and
# transpose PE cycles at ~2e-3 output error (vs ~2e-4).
GATHER_BF16 = os.environ.get("KGATHER_BF16", "0") == "1"
g_dt = bf16 if GATHER_BF16 else f32
G_ELEM = 64 if GATHER_BF16 else 128  # dma_gather elem_size: bf16 rows read as 128 x 2B


def _class_blocks(cap):
    """Block sizes (k = column-blocks) for one class capacity (multiple of 128)."""
    n = cap // P
    out = [K_FULL] * (n // K_FULL)
    if n % K_FULL:
        out.append(n % K_FULL)
    return out


def _build_module(blocks):
    """blocks: list of (k, hi0, hi1) — identical on every core."""
    sb_dt = {"bf16": bf16, "f32r": f32r, "f32": f32}[MM_MODE]
    ecp = sum(k for k, _, _ in blocks) * P
    idx_tot = sum(P * 8 * k for k, _, _ in blocks)  # per-block [P, 8k] int16

    nc = bacc.Bacc("TRN2", num_swdge_queues=4)
    xn = nc.dram_tensor("x_node", [N_NODES, D], g_dt, kind="ExternalInput")
    xe_dt = bf16 if GATHER_BF16 else f32r
    xet_d = nc.dram_tensor("x_edgeT", [ecp * D], xe_dt, kind="ExternalInput")
    i0 = nc.dram_tensor("idx0", [idx_tot], i16, kind="ExternalInput")
    i1 = nc.dram_tensor("idx1", [idx_tot], i16, kind="ExternalInput")
    w1 = nc.dram_tensor("W1", [3 * D, D], f32, kind="ExternalInput")
    b1 = nc.dram_tensor("b1", [D], f32, kind="ExternalInput")
    w2 = nc.dram_tensor("W2", [D, D], f32, kind="ExternalInput")
    b2r = nc.dram_tensor("b2r", [P, D], f32, kind="ExternalInput")
    out = nc.dram_tensor("out", [ecp, D], f32, kind="ExternalOutput")

    xn_hi = xn[TH:, :]

    from concourse.masks import make_identity

    with (
        tile.TileContext(nc) as tc,
        tc.tile_pool(name="const", bufs=1) as cpool,
        tc.tile_pool(name="big", bufs=3) as big,
        tc.tile_pool(name="tsp", bufs=2) as tsp,
        tc.tile_pool(name="psT", bufs=1, space="PSUM") as psT,
        tc.tile_pool(name="psH", bufs=2, space="PSUM") as psH,
        tc.tile_pool(name="psO", bufs=2, space="PSUM") as psO,
    ):
        ident = cpool.tile([P, P], g_dt)
        make_identity(nc, ident[:])
        mm01_dt = bf16 if GATHER_BF16 else sb_dt

        # W1 as 3 K-chunks: chunk c = W1[128c:128c+128, :] -> w1_sb[:, c, :]
        # Staged through fp32 then copied so the compute-dtype rounding is
        # done by an engine write (required for FP32R).
        w1_sb = cpool.tile([P, 3, D], sb_dt)
        w2_sb = cpool.tile([P, D], sb_dt)
        if MM_MODE == "f32":
            nc.sync.dma_start(out=w1_sb[:], in_=w1[:].rearrange("(c p) d -> p c d", p=P))
            nc.sync.dma_start(out=w2_sb[:], in_=w2[:])
        else:
            w1_st = cpool.tile([P, 3, D], f32)
            w2_st = cpool.tile([P, D], f32)
            nc.sync.dma_start(out=w1_st[:], in_=w1[:].rearrange("(c p) d -> p c d", p=P))
            nc.sync.dma_start(out=w2_st[:], in_=w2[:])
            nc.any.tensor_copy(w1_sb[:], w1_st[:])
            nc.any.tensor_copy(w2_sb[:], w2_st[:])
        w1_01 = cpool.tile([P, 3, D], mm01_dt)
        if MM_MODE == "f32":
            w1_f32src = w1_sb
        else:
            w1_f32src = w1_st
        nc.any.tensor_copy(w1_01[:], w1_f32src[:])
        b1_sb = cpool.tile([P, 1], f32)
        nc.sync.dma_start(out=b1_sb[:], in_=b1[:].unsqueeze(1))
        # b2 replicated across partitions, prepared host-side.
        b2bc = cpool.tile([P, D], f32)
        nc.sync.dma_start(out=b2bc[:], in_=b2r[:])

        eoff = 0
        ioff = 0
        goff = 0
        gq = 0
        for kb, hi0, hi1 in blocks:
            BL = P * kb
            S = 8 * kb  # int16 index free dim: ceil(BL/16)
            v0 = big.tile([P, kb * D], g_dt, tag="v0")
            v1 = big.tile([P, kb * D], g_dt, tag="v1")
            xbt = big.tile([P, kb * D], xe_dt, tag="xbt")
            ot = big.tile([P, kb * D], f32, tag="ot")
            ix0 = big.tile([P, S], i16, tag="ix0")
            ix1 = big.tile([P, S], i16, tag="ix1")

            nc.sync.dma_start(out=ix0[:], in_=i0[ioff : ioff + P * S].rearrange("(p s) -> p s", p=P))
            nc.sync.dma_start(out=ix1[:], in_=i1[ioff : ioff + P * S].rearrange("(p s) -> p s", p=P))
            nc.sync.dma_start(
                out=xbt[:],
                in_=xet_d[goff : goff + BL * D].rearrange("(p n) -> p n", p=P),
            )
            # dma_gather crashes the device above ~1024 indices per
            # instruction; chunk by 8 col-blocks.
            for vt, ixt, hi in ((v0, ix0, hi0), (v1, ix1, hi1)):
                for k0 in range(0, kb, 8):
                    kw = min(8, kb - k0)
                    nc.gpsimd.dma_gather(
                        vt[:, k0 * D : (k0 + kw) * D].rearrange("p (k d) -> p k d", d=D),
                        xn_hi if hi else xn[:, :],
                        ixt[:, 8 * k0 : 8 * (k0 + kw)],
                        P * kw,
                        P * kw,
                        D,
                        queue_num=gq % 4,
                    )
                    gq += 1

            for g0 in range(0, kb, GROUP):
                wid = min(GROUP, kb - g0)
                N = wid * P

                ph = psH.tile([P, GROUP * P], f32, tag="ph")
                for kind, src in enumerate((v0, v1)):
                    pv = psT.tile([P, GROUP * P], g_dt, tag=f"pv{kind}", name=f"pv{kind}")
                    tv = tsp.tile([P, GROUP * P], mm01_dt, tag=f"tv{kind}", name=f"tv{kind}")
                    for c in range(wid):
                        j = g0 + c
                        nc.tensor.transpose(
                            out=pv[:, c * P : (c + 1) * P],
                            in_=src[:, j * D : (j + 1) * D],
                            identity=ident[:],
                        )
                    # Fixed engine per kind: v0 copy on ACT, v1 on DVE.
                    if kind == 0:
                        nc.scalar.activation(
                            out=tv[:, :N], in_=pv[:, :N],
                            func=mybir.ActivationFunctionType.Copy,
                        )
                    else:
                        nc.vector.tensor_copy(tv[:, :N], pv[:, :N])
                    nc.tensor.matmul(
                        ph[:, :N],
                        lhsT=w1_01[:, kind, :],
                        rhs=tv[:, :N],
                        start=(kind == 0),
                        stop=False,
                    )
                nc.tensor.matmul(
                    ph[:, :N],
                    lhsT=(w1_01 if GATHER_BF16 else w1_sb)[:, 2, :],
                    rhs=xbt[:, g0 * P : g0 * P + N],
                    start=False,
                    stop=True,
                )

                h = tsp.tile([P, GROUP * P], sb_dt, tag="h")
                nc.scalar.activation(
                    out=h[:, :N],
                    in_=ph[:, :N],
                    func=mybir.ActivationFunctionType.Relu,
                    bias=b1_sb[:, 0:1],
                )

                po = psO.tile([P, GROUP * P], f32, tag="po")
                for c in range(wid):
                    nc.tensor.matmul(
                        po[:, c * P : (c + 1) * P],
                        lhsT=h[:, c * P : (c + 1) * P],
                        rhs=w2_sb[:],
                        start=True,
                        stop=True,
                    )

                # out = po + b2 (broadcast along edges), PSUM -> SBUF
                nc.vector.tensor_tensor(
                    out=ot[:, g0 * D : g0 * D + N].rearrange("p (c d) -> p c d", d=P),
                    in0=po[:, :N].rearrange("p (c d) -> p c d", d=P),
                    in1=b2bc[:].unsqueeze(1).to_broadcast([P, wid, P]),
                    op=mybir.AluOpType.add,
                )

            nc.scalar.dma_start(
                out=out[eoff : eoff + BL, :].rearrange("(p k) d -> p (k d)", p=P),
                in_=ot[:],
            )
            eoff += BL
            ioff += P * S
            goff += BL * D

    nc.compile()
    return nc


def _pack_idx(vals):
    """[BL] int16 values -> flat [P*8k] device layout: idx i at
    (partition i%16 replicated 8x, free slot i//16), partition-major."""
    BL = vals.shape[0]
    S = BL // 16
    t16 = vals.reshape(S, 16).T  # [16, S]
    return np.tile(t16, (8, 1)).ravel()  # [128, S] -> flat p-major


def _plan_and_pack(x_edge, ei):
    """Host-side: class-partition, permute, build per-core device arrays.

    Returns (blocks, per-core input dicts, per-core (perm, valid), ecp)."""
    v0 = ei[:, 0].astype(np.int64)
    v1 = ei[:, 1].astype(np.int64)
    cls = (v0 >= TH) * 2 + (v1 >= TH)

    per_core = []
    for c in range(N_CORES):
        sl = slice(c * EC, (c + 1) * EC)
        per_core.append((v0[sl], v1[sl], cls[sl]))

    caps = []
    for cl in range(4):
        cnt = max(int((pc[2] == cl).sum()) for pc in per_core)
        caps.append(-(-max(cnt, 1) // P) * P)

    blocks = []
    for cl in range(4):
        hi0, hi1 = bool(cl & 2), bool(cl & 1)
        blocks.extend((k, hi0, hi1) for k in _class_blocks(caps[cl]))
    ecp = sum(k for k, _, _ in blocks) * P

    core_data = []
    core_asm = []
    for c in range(N_CORES):
        cv0, cv1, ccls = per_core[c]
        xe_core = x_edge[c * EC : (c + 1) * EC]

        # permuted order: class-grouped, padded per class
        perm = np.full(ecp, -1, dtype=np.int64)  # padded-perm pos -> core-local edge
        off = 0
        for cl in range(4):
            ids = np.nonzero(ccls == cl)[0]
            # ascending v0 within the class: gather addresses mostly
            # monotonic -> better HBM locality for the v0 gather
            ids = ids[np.argsort(cv0[ids], kind="stable")]
            perm[off : off + len(ids)] = ids
            off += caps[cl]
        valid = perm >= 0
        pidx = np.where(valid, perm, 0)

        pos = np.arange(ecp)
        hi0_mask = pos >= caps[0] + caps[1]
        hi1_mask = ((pos >= caps[0]) & (pos < caps[0] + caps[1])) | (
            pos >= caps[0] + caps[1] + caps[2]
        )
        pv0 = np.where(valid, cv0[pidx] - np.where(hi0_mask, TH, 0), 0).astype(np.int16)
        pv1 = np.where(valid, cv1[pidx] - np.where(hi1_mask, TH, 0), 0).astype(np.int16)

        xe_perm = np.zeros((ecp, D), np.float32)
        xe_perm[valid] = xe_core[perm[valid]]

        # xeT: per matmul group, the [N, D] edge slab transposed to [D, N]
        # (feature-major) so the device loads mm1's rhs directly.
        xeT_parts = []
        i0_parts = []
        i1_parts = []
        eoff = 0
        for kb, _, _ in blocks:
            BL = P * kb
            slab = xe_perm[eoff : eoff + BL]  # [BL, D], edge i = c*128+p
            xeT_parts.append(np.ascontiguousarray(slab.T).ravel())
            i0_parts.append(_pack_idx(pv0[eoff : eoff + BL]))
            i1_parts.append(_pack_idx(pv1[eoff : eoff + BL]))
            eoff += BL

        core_data.append(
            {
                "x_edgeT": np.concatenate(xeT_parts),
                "idx0": np.concatenate(i0_parts),
                "idx1": np.concatenate(i1_parts),
            }
        )
        core_asm.append((perm, valid))

    return blocks, core_data, core_asm, ecp


_module_cache = {}


def _get_module(blocks_key):
    if blocks_key not in _module_cache:
        _module_cache[blocks_key] = _build_module(list(blocks_key))
    return _module_cache[blocks_key]


def run(inputs, trace=False):
    """Run on 8 cores. Returns (full_output [E, D] fp32, BassKernelResults)."""
    x_node = np.ascontiguousarray(np.asarray(inputs["x_node"], dtype=np.float32))
    x_edge = np.ascontiguousarray(np.asarray(inputs["x_edge"], dtype=np.float32))
    ei = np.asarray(inputs["edge_index"])
    W1 = np.ascontiguousarray(np.asarray(inputs["W1"], dtype=np.float32))
    b1 = np.ascontiguousarray(np.asarray(inputs["b1"], dtype=np.float32))
    W2 = np.ascontiguousarray(np.asarray(inputs["W2"], dtype=np.float32))
    b2 = np.ascontiguousarray(np.asarray(inputs["b2"], dtype=np.float32))
    b2r_host = np.ascontiguousarray(np.broadcast_to(b2, (P, D)).copy())

    blocks, core_data, core_asm, ecp = _plan_and_pack(x_edge, ei)
    nc = _get_module(tuple(blocks))

    if GATHER_BF16:
        import ml_dtypes

        x_node = x_node.astype(ml_dtypes.bfloat16)
        for cd in core_data:
            cd["x_edgeT"] = cd["x_edgeT"].astype(ml_dtypes.bfloat16)

    in_maps = []
    for c in range(N_CORES):
        in_maps.append(
            {
                "x_node": x_node,
                "x_edgeT": core_data[c]["x_edgeT"],
                "idx0": core_data[c]["idx0"],
                "idx1": core_data[c]["idx1"],
                "W1": W1,
                "b1": b1,
                "W2": W2,
                "b2r": b2r_host,
            }
        )

    res = run_bass_kernel_spmd(nc, in_maps, core_ids=list(range(N_CORES)), trace=trace)

    full = np.empty((E_TOTAL, D), np.float32)
    eoffs = np.cumsum([0] + [P * k for k, _, _ in blocks])
    for c in range(N_CORES):
        dev_out = res.results[c]["out"]
        perm, valid = core_asm[c]
        res_perm = np.empty((ecp, D), np.float32)
        for bi, (kb, _, _) in enumerate(blocks):
            a, b = int(eoffs[bi]), int(eoffs[bi + 1])
            res_perm[a:b] = (
                dev_out[a:b].reshape(P, kb, D).transpose(1, 0, 2).reshape(b - a, D)
            )
        out_core = full[c * EC : (c + 1) * EC]
        out_core[perm[valid]] = res_perm[valid]
    return full, res


def kernel(**inputs):
    out, _ = run(inputs, trace=False)
    return out


# revision 20
# speedup vs baseline: 1.4326x; 1.0047x over previous
"""Trainium2 Bass kernel for an edge-MLP GNN block.

  v0 = x_node[edge_index[:, 0]]          # [E, D] gather
  v1 = x_node[edge_index[:, 1]]          # [E, D] gather
  h  = relu(concat([v0, v1, x_edge]) @ W1 + b1)
  out = h @ W2 + b2                      # [E, D]

Sharding: edges are split evenly across 8 NeuronCores; x_node and the MLP
weights are replicated.  No cross-core communication.

Gather: the SWDGE `dma_gather` extended instruction fetches one 512B node row
per index, thousands of rows per instruction (descriptor-generation is the
per-instruction cost, ~1us + 0.34ns/row).  Its indices are int16, so the host
partitions each core's edges into 4 classes by (src >= 32768, dst >= 32768)
and each block gathers from a base-offset view of x_node; the host permutes
edges into class-contiguous order and inverse-permutes the output.

Per-core layout: edges are processed in blocks of 128*k edges.  dma_gather
writes row i of a block to partition i%128, column-block i//128.  The host
pre-permutes the x_edge shard (and inverse-permutes the output) so the device
x_edge load and output store stay fully contiguous per partition.

On-chip per 512-edge group: PE transposes the three [128e x 128f] operand
tiles into feature-major form, mm1 accumulates the three K-chunks of W1 into
PSUM ([d1 x 512e]), ACT applies bias+ReLU, mm2 multiplies each 128-edge chunk
by W2 (output lands edge-major, no output transpose needed), and DVE adds b2
while copying PSUM->SBUF.
"""

import os
import sys

sys.path.insert(0, "/opt/trn_rl_repo")

import numpy as np

import concourse.bacc as bacc
import concourse.bass as bass
import concourse.mybir as mybir
import concourse.tile as tile
from concourse.bass_utils import run_bass_kernel_spmd

P = 128
D = 128
N_NODES = 50000
E_TOTAL = 500000
N_CORES = 8
EC = E_TOTAL // N_CORES  # 62500 edges per core
TH = 32768               # int16 index ceiling; class split threshold

K_FULL = 16              # column-blocks (edges per partition) per full block
GROUP = 4                # column-blocks per matmul group (N = 512)

f32 = mybir.dt.float32
f32r = mybir.dt.float32r
bf16 = mybir.dt.bfloat16
i16 = mybir.dt.int16

# Compute dtype for the matmuls: "f32" (exact, 4 cyc/row), "f32r" (single-pass
# fp32, 1 cyc/row at N>=256), "bf16" (1 cyc/row; activations/weights rounded).
MM_MODE = os.environ.get("KMM_DTYPE", "f32r")
# Gather node rows + x_edge in bf16 (host-cast): halves their DMA bytes and
# transpose PE cycles. Output rel err ~2.4e-3 (vs ~2e-4 with KGATHER_BF16=0
# at ~1.4x the runtime).
GATHER_BF16 = os.environ.get("KGATHER_BF16", "1") == "1"
g_dt = bf16 if GATHER_BF16 else f32
G_ELEM = 64 if GATHER_BF16 else 128  # dma_gather elem_size: bf16 rows read as 128 x 2B


def _class_blocks(cap):
    """Block sizes (k = column-blocks) for one class capacity (multiple of 128)."""
    n = cap // P
    out = [K_FULL] * (n // K_FULL)
    if n % K_FULL:
        out.append(n % K_FULL)
    return out


def _build_module(blocks):
    """blocks: list of (k, hi0, hi1) — identical on every core."""
    sb_dt = {"bf16": bf16, "f32r": f32r, "f32": f32}[MM_MODE]
    ecp = sum(k for k, _, _ in blocks) * P
    idx_tot = sum(P * 8 * k for k, _, _ in blocks)  # per-block [P, 8k] int16

    nc = bacc.Bacc("TRN2", num_swdge_queues=4)
    xn = nc.dram_tensor("x_node", [N_NODES, D], g_dt, kind="ExternalInput")
    xe_dt = bf16 if GATHER_BF16 else f32r
    xet_d = nc.dram_tensor("x_edgeT", [ecp * D], xe_dt, kind="ExternalInput")
    i0 = nc.dram_tensor("idx0", [idx_tot], i16, kind="ExternalInput")
    i1 = nc.dram_tensor("idx1", [idx_tot], i16, kind="ExternalInput")
    w1 = nc.dram_tensor("W1", [3 * D, D], f32, kind="ExternalInput")
    b1 = nc.dram_tensor("b1", [D], f32, kind="ExternalInput")
    w2 = nc.dram_tensor("W2", [D, D], f32, kind="ExternalInput")
    b2r = nc.dram_tensor("b2r", [P, D], f32, kind="ExternalInput")
    out = nc.dram_tensor("out", [ecp, D], f32, kind="ExternalOutput")

    xn_hi = xn[TH:, :]

    from concourse.masks import make_identity

    with (
        tile.TileContext(nc) as tc,
        tc.tile_pool(name="const", bufs=1) as cpool,
        tc.tile_pool(name="big", bufs=3) as big,
        tc.tile_pool(name="tsp", bufs=2) as tsp,
        tc.tile_pool(name="psT", bufs=1, space="PSUM") as psT,
        tc.tile_pool(name="psH", bufs=2, space="PSUM") as psH,
        tc.tile_pool(name="psO", bufs=2, space="PSUM") as psO,
    ):
        ident = cpool.tile([P, P], g_dt)
        make_identity(nc, ident[:])
        mm01_dt = bf16 if GATHER_BF16 else sb_dt

        # W1 as 3 K-chunks: chunk c = W1[128c:128c+128, :] -> w1_sb[:, c, :]
        # Staged through fp32 then copied so the compute-dtype rounding is
        # done by an engine write (required for FP32R).
        w1_sb = cpool.tile([P, 3, D], sb_dt)
        w2_sb = cpool.tile([P, D], sb_dt)
        if MM_MODE == "f32":
            nc.sync.dma_start(out=w1_sb[:], in_=w1[:].rearrange("(c p) d -> p c d", p=P))
            nc.sync.dma_start(out=w2_sb[:], in_=w2[:])
        else:
            w1_st = cpool.tile([P, 3, D], f32)
            w2_st = cpool.tile([P, D], f32)
            nc.sync.dma_start(out=w1_st[:], in_=w1[:].rearrange("(c p) d -> p c d", p=P))
            nc.sync.dma_start(out=w2_st[:], in_=w2[:])
            nc.any.tensor_copy(w1_sb[:], w1_st[:])
            nc.any.tensor_copy(w2_sb[:], w2_st[:])
        w1_01 = cpool.tile([P, 3, D], mm01_dt)
        if MM_MODE == "f32":
            w1_f32src = w1_sb
        else:
            w1_f32src = w1_st
        nc.any.tensor_copy(w1_01[:], w1_f32src[:])
        b1_sb = cpool.tile([P, 1], f32)
        nc.sync.dma_start(out=b1_sb[:], in_=b1[:].unsqueeze(1))
        # b2 replicated across partitions, prepared host-side.
        b2bc = cpool.tile([P, D], f32)
        nc.sync.dma_start(out=b2bc[:], in_=b2r[:])

        eoff = 0
        ioff = 0
        goff = 0
        gq = 0
        for kb, hi0, hi1 in blocks:
            BL = P * kb
            S = 8 * kb  # int16 index free dim: ceil(BL/16)
            v0 = big.tile([P, kb * D], g_dt, tag="v0")
            v1 = big.tile([P, kb * D], g_dt, tag="v1")
            xbt = big.tile([P, kb * D], xe_dt, tag="xbt")
            ot = big.tile([P, kb * D], f32, tag="ot")
            ix0 = big.tile([P, S], i16, tag="ix0")
            ix1 = big.tile([P, S], i16, tag="ix1")

            nc.sync.dma_start(out=ix0[:], in_=i0[ioff : ioff + P * S].rearrange("(p s) -> p s", p=P))
            nc.sync.dma_start(out=ix1[:], in_=i1[ioff : ioff + P * S].rearrange("(p s) -> p s", p=P))
            nc.sync.dma_start(
                out=xbt[:],
                in_=xet_d[goff : goff + BL * D].rearrange("(p n) -> p n", p=P),
            )
            # dma_gather crashes the device above ~1024 indices per
            # instruction; chunk by 8 col-blocks.
            for vt, ixt, hi in ((v0, ix0, hi0), (v1, ix1, hi1)):
                for k0 in range(0, kb, 8):
                    kw = min(8, kb - k0)
                    nc.gpsimd.dma_gather(
                        vt[:, k0 * D : (k0 + kw) * D].rearrange("p (k d) -> p k d", d=D),
                        xn_hi if hi else xn[:, :],
                        ixt[:, 8 * k0 : 8 * (k0 + kw)],
                        P * kw,
                        P * kw,
                        D,
                        queue_num=gq % 4,
                    )
                    gq += 1

            for g0 in range(0, kb, GROUP):
                wid = min(GROUP, kb - g0)
                N = wid * P

                ph = psH.tile([P, GROUP * P], f32, tag="ph")
                for kind, src in enumerate((v0, v1)):
                    pv = psT.tile([P, GROUP * P], g_dt, tag=f"pv{kind}", name=f"pv{kind}")
                    tv = tsp.tile([P, GROUP * P], mm01_dt, tag=f"tv{kind}", name=f"tv{kind}")
                    for c in range(wid):
                        j = g0 + c
                        nc.tensor.transpose(
                            out=pv[:, c * P : (c + 1) * P],
                            in_=src[:, j * D : (j + 1) * D],
                            identity=ident[:],
                        )
                    # Fixed engine per kind: v0 copy on ACT, v1 on DVE.
                    if kind == 0:
                        nc.scalar.activation(
                            out=tv[:, :N], in_=pv[:, :N],
                            func=mybir.ActivationFunctionType.Copy,
                        )
                    else:
                        nc.vector.tensor_copy(tv[:, :N], pv[:, :N])
                    nc.tensor.matmul(
                        ph[:, :N],
                        lhsT=w1_01[:, kind, :],
                        rhs=tv[:, :N],
                        start=(kind == 0),
                        stop=False,
                    )
                nc.tensor.matmul(
                    ph[:, :N],
                    lhsT=(w1_01 if GATHER_BF16 else w1_sb)[:, 2, :],
                    rhs=xbt[:, g0 * P : g0 * P + N],
                    start=False,
                    stop=True,
                )

                h = tsp.tile([P, GROUP * P], sb_dt, tag="h")
                nc.scalar.activation(
                    out=h[:, :N],
                    in_=ph[:, :N],
                    func=mybir.ActivationFunctionType.Relu,
                    bias=b1_sb[:, 0:1],
                )

                po = psO.tile([P, GROUP * P], f32, tag="po")
                for c in range(wid):
                    nc.tensor.matmul(
                        po[:, c * P : (c + 1) * P],
                        lhsT=h[:, c * P : (c + 1) * P],
                        rhs=w2_sb[:],
                        start=True,
                        stop=True,
                    )

                # out = po + b2 (broadcast along edges), PSUM -> SBUF
                nc.vector.tensor_tensor(
                    out=ot[:, g0 * D : g0 * D + N].rearrange("p (c d) -> p c d", d=P),
                    in0=po[:, :N].rearrange("p (c d) -> p c d", d=P),
                    in1=b2bc[:].unsqueeze(1).to_broadcast([P, wid, P]),
                    op=mybir.AluOpType.add,
                )

            nc.scalar.dma_start(
                out=out[eoff : eoff + BL, :].rearrange("(p k) d -> p (k d)", p=P),
                in_=ot[:],
            )
            eoff += BL
            ioff += P * S
            goff += BL * D

    nc.compile()
    return nc


def _pack_idx(vals):
    """[BL] int16 values -> flat [P*8k] device layout: idx i at
    (partition i%16 replicated 8x, free slot i//16), partition-major."""
    BL = vals.shape[0]
    S = BL // 16
    t16 = vals.reshape(S, 16).T  # [16, S]
    return np.tile(t16, (8, 1)).ravel()  # [128, S] -> flat p-major


def _plan_and_pack(x_edge, ei):
    """Host-side: class-partition, permute, build per-core device arrays.

    Returns (blocks, per-core input dicts, per-core (perm, valid), ecp)."""
    v0 = ei[:, 0].astype(np.int64)
    v1 = ei[:, 1].astype(np.int64)
    cls = (v0 >= TH) * 2 + (v1 >= TH)

    per_core = []
    for c in range(N_CORES):
        sl = slice(c * EC, (c + 1) * EC)
        per_core.append((v0[sl], v1[sl], cls[sl]))

    caps = []
    for cl in range(4):
        cnt = max(int((pc[2] == cl).sum()) for pc in per_core)
        caps.append(-(-max(cnt, 1) // P) * P)

    blocks = []
    for cl in range(4):
        hi0, hi1 = bool(cl & 2), bool(cl & 1)
        blocks.extend((k, hi0, hi1) for k in _class_blocks(caps[cl]))
    ecp = sum(k for k, _, _ in blocks) * P

    core_data = []
    core_asm = []
    for c in range(N_CORES):
        cv0, cv1, ccls = per_core[c]
        xe_core = x_edge[c * EC : (c + 1) * EC]

        # permuted order: class-grouped, padded per class
        perm = np.full(ecp, -1, dtype=np.int64)  # padded-perm pos -> core-local edge
        off = 0
        for cl in range(4):
            ids = np.nonzero(ccls == cl)[0]
            # ascending v0 within the class: gather addresses mostly
            # monotonic -> better HBM locality for the v0 gather
            ids = ids[np.argsort(cv0[ids], kind="stable")]
            perm[off : off + len(ids)] = ids
            off += caps[cl]
        valid = perm >= 0
        pidx = np.where(valid, perm, 0)

        pos = np.arange(ecp)
        hi0_mask = pos >= caps[0] + caps[1]
        hi1_mask = ((pos >= caps[0]) & (pos < caps[0] + caps[1])) | (
            pos >= caps[0] + caps[1] + caps[2]
        )
        pv0 = np.where(valid, cv0[pidx] - np.where(hi0_mask, TH, 0), 0).astype(np.int16)
        pv1 = np.where(valid, cv1[pidx] - np.where(hi1_mask, TH, 0), 0).astype(np.int16)

        xe_perm = np.zeros((ecp, D), np.float32)
        xe_perm[valid] = xe_core[perm[valid]]

        # xeT: per matmul group, the [N, D] edge slab transposed to [D, N]
        # (feature-major) so the device loads mm1's rhs directly.
        xeT_parts = []
        i0_parts = []
        i1_parts = []
        eoff = 0
        for kb, _, _ in blocks:
            BL = P * kb
            slab = xe_perm[eoff : eoff + BL]  # [BL, D], edge i = c*128+p
            xeT_parts.append(np.ascontiguousarray(slab.T).ravel())
            i0_parts.append(_pack_idx(pv0[eoff : eoff + BL]))
            i1_parts.append(_pack_idx(pv1[eoff : eoff + BL]))
            eoff += BL

        core_data.append(
            {
                "x_edgeT": np.concatenate(xeT_parts),
                "idx0": np.concatenate(i0_parts),
                "idx1": np.concatenate(i1_parts),
            }
        )
        core_asm.append((perm, valid))

    return blocks, core_data, core_asm, ecp


_module_cache = {}


def _get_module(blocks_key):
    if blocks_key not in _module_cache:
        _module_cache[blocks_key] = _build_module(list(blocks_key))
    return _module_cache[blocks_key]


def run(inputs, trace=False):
    """Run on 8 cores. Returns (full_output [E, D] fp32, BassKernelResults)."""
    x_node = np.ascontiguousarray(np.asarray(inputs["x_node"], dtype=np.float32))
    x_edge = np.ascontiguousarray(np.asarray(inputs["x_edge"], dtype=np.float32))
    ei = np.asarray(inputs["edge_index"])
    W1 = np.ascontiguousarray(np.asarray(inputs["W1"], dtype=np.float32))
    b1 = np.ascontiguousarray(np.asarray(inputs["b1"], dtype=np.float32))
    W2 = np.ascontiguousarray(np.asarray(inputs["W2"], dtype=np.float32))
    b2 = np.ascontiguousarray(np.asarray(inputs["b2"], dtype=np.float32))
    b2r_host = np.ascontiguousarray(np.broadcast_to(b2, (P, D)).copy())

    blocks, core_data, core_asm, ecp = _plan_and_pack(x_edge, ei)
    nc = _get_module(tuple(blocks))

    if GATHER_BF16:
        import ml_dtypes

        x_node = x_node.astype(ml_dtypes.bfloat16)
        for cd in core_data:
            cd["x_edgeT"] = cd["x_edgeT"].astype(ml_dtypes.bfloat16)

    in_maps = []
    for c in range(N_CORES):
        in_maps.append(
            {
                "x_node": x_node,
                "x_edgeT": core_data[c]["x_edgeT"],
                "idx0": core_data[c]["idx0"],
                "idx1": core_data[c]["idx1"],
                "W1": W1,
                "b1": b1,
                "W2": W2,
                "b2r": b2r_host,
            }
        )

    res = run_bass_kernel_spmd(nc, in_maps, core_ids=list(range(N_CORES)), trace=trace)

    full = np.empty((E_TOTAL, D), np.float32)
    eoffs = np.cumsum([0] + [P * k for k, _, _ in blocks])
    for c in range(N_CORES):
        dev_out = res.results[c]["out"]
        perm, valid = core_asm[c]
        res_perm = np.empty((ecp, D), np.float32)
        for bi, (kb, _, _) in enumerate(blocks):
            a, b = int(eoffs[bi]), int(eoffs[bi + 1])
            res_perm[a:b] = (
                dev_out[a:b].reshape(P, kb, D).transpose(1, 0, 2).reshape(b - a, D)
            )
        out_core = full[c * EC : (c + 1) * EC]
        out_core[perm[valid]] = res_perm[valid]
    return full, res


def kernel(**inputs):
    out, _ = run(inputs, trace=False)
    return out
